# revision 22
# baseline (speedup 1.0000x reference)
"""BertSelfAttention Trainium2 kernel.

Shapes: hidden_states [S=1024, B=4, D=1024], H=16 heads of DH=64.
Sharding: 2 heads per core (8 cores). Each core receives the full hidden
states (pre-transposed + bf16-cast on host) and a 128-row slice of each
projection weight, computes the full attention chain for its two heads with
no cross-core communication, and writes ctx^T per (batch, head).

Device-side layout tricks:
  - scores are computed transposed (scoresT[u, t] = q_t . k_u) so the
    additive attention mask (per key position u) is a per-partition bias
    that fuses into the Exp activation: probsT = exp(scores/8 + mask).
  - V carries an appended ones-column, so the AV matmul produces the
    softmax denominator in row DH of ctxT for free.
  - normalization multiplies ctxT rows by reciprocal(denom) broadcast
    across partitions via a K=1 ones matmul.
"""

import os
import numpy as np
import ml_dtypes

S, B, D, H = 1024, 4, 1024, 16
DH = D // H          # 64
NCORES = 8
HPC = H // NCORES    # heads per core = 2
P = 128              # partitions / d-tile / t-tile
DCH = D // P         # 8 contraction tiles
BS = B * S           # 4096 flattened (b, s)
CH = 512             # matmul free-dim chunk (fp32 rhs limit)

_compiled_nc = None
last_exec_time_ns = None
last_results = None


def _build():
    import concourse.bacc as bacc
    import concourse.bass as bass
    import concourse.mybir as mybir
    import concourse.tile as tile
    from contextlib import ExitStack

    f32 = mybir.dt.float32
    bf16 = mybir.dt.bfloat16
    AF = mybir.ActivationFunctionType

    nc = bacc.Bacc("TRN2", target_bir_lowering=False, debug=False,
                   num_devices=NCORES)

    hT_d = nc.dram_tensor("hT", [D, BS], bf16, kind="ExternalInput")
    wqT_d = nc.dram_tensor("wqT", [D, P], bf16, kind="ExternalInput")
    wkT_d = nc.dram_tensor("wkT", [D, P], bf16, kind="ExternalInput")
    wvT_d = nc.dram_tensor("wvT", [D, P], bf16, kind="ExternalInput")
    # packed per-partition constants: [bq | bk | bvb(128) | maskT(8*4)]
    misc_d = nc.dram_tensor("misc", [P, 2 + P + DCH * B], f32,
                            kind="ExternalInput")
    out_d = nc.dram_tensor("out", [B, HPC, DH, S], f32, kind="ExternalOutput")

    with tile.TileContext(nc) as tc, ExitStack() as ctx:
        persist = ctx.enter_context(tc.tile_pool(name="persist", bufs=1))
        probs_pool = ctx.enter_context(tc.tile_pool(name="probs", bufs=34))
        small = ctx.enter_context(tc.tile_pool(name="small", bufs=2))
        out_pool = ctx.enter_context(tc.tile_pool(name="outp", bufs=4))
        ps_mm = ctx.enter_context(tc.tile_pool(name="ps_mm", bufs=2, space="PSUM"))
        ps_sc = ctx.enter_context(tc.tile_pool(name="ps_sc", bufs=2, space="PSUM"))
        ps_ctx = ctx.enter_context(tc.tile_pool(name="ps_ctx", bufs=2, space="PSUM"))

        # ---- persistent SBUF tensors ----
        hT_sb = persist.tile([P, DCH, BS], bf16)        # hidden^T, d-tiled
        wq_sb = persist.tile([P, DCH, P], bf16)
        wk_sb = persist.tile([P, DCH, P], bf16)
        wv_sb = persist.tile([P, DCH, P], bf16)
        misc_sb = persist.tile([P, 2 + P + DCH * B], f32)
        qT_sb = persist.tile([P, BS], bf16)             # Q^T [i, t]
        kT_sb = persist.tile([P, BS], bf16)             # K^T [i, t]
        # V in [t, j] layout + ones column per head: [t-part, t-tile, head, DH+1]
        v_sb = persist.tile([P, BS // P, HPC, DH + 1], bf16)
        dummy_sb = persist.tile([P, CH], bf16)

        bq_sb = misc_sb[:, 0:1]
        bk_sb = misc_sb[:, 1:2]
        bvb_sb = misc_sb[:, 2:2 + P]

        def mask_bias(uc, bi):
            c = 2 + P + uc * B + bi
            return misc_sb[:, c:c + 1]

        # ---- HAM warmup: dead matmuls keep the PE busy while inputs load,
        # so the real work starts at the 2.4 GHz clock.
        nc.vector.memset(dummy_sb[:], 0.0)
        for _ in range(24):
            d_ps = ps_sc.tile([P, CH], f32, tag="sc", name="d_ps")
            nc.tensor.matmul(d_ps[:], dummy_sb[:, 0:P], dummy_sb[:],
                             start=True, stop=True)

        # ---- input DMAs ----
        # small tensors first so projections can start immediately; hT in
        # per-batch column pieces (dc-minor) so batch 0's full contraction
        # input lands in the first few microseconds. Pieces alternate
        # between the two HWDGE queues (SP / ACT) to overlap.
        nc.sync.dma_start(wq_sb[:], wqT_d.ap().rearrange("(dc p) m -> p dc m", p=P))
        nc.scalar.dma_start(wk_sb[:], wkT_d.ap().rearrange("(dc p) m -> p dc m", p=P))
        nc.sync.dma_start(wv_sb[:], wvT_d.ap().rearrange("(dc p) m -> p dc m", p=P))
        nc.scalar.dma_start(misc_sb[:], misc_d.ap())
        hT_re = hT_d.ap().rearrange("(dc p) t -> p dc t", p=P)
        k = 0
        for q in range(B):
            qsl = slice(q * S, (q + 1) * S)
            for dc in range(DCH):
                eng = nc.sync if k % 2 == 0 else nc.scalar
                eng.dma_start(hT_sb[:, dc, qsl], hT_re[:, dc, qsl])
                k += 1

        nc.vector.memset(v_sb[:, :, :, DH:DH + 1], 1.0)

        scale = 1.0 / float(np.sqrt(DH))

        def emit_qk_chunk(w_sb, b_sb, dst, ci, pool):
            sl = slice(ci * CH, (ci + 1) * CH)
            qk_ps = pool.tile([P, CH], f32, tag="mm" if pool is ps_mm else "sc",
                              name="qk_ps")
            for dc in range(DCH):
                nc.tensor.matmul(
                    qk_ps[:], w_sb[:, dc, :], hT_sb[:, dc, sl],
                    start=(dc == 0), stop=(dc == DCH - 1))
            nc.vector.tensor_scalar_add(dst[:, sl], qk_ps[:], b_sb[:])

        def emit_v_tile(tt, pool):
            tsl = slice(tt * P, (tt + 1) * P)
            v_ps = pool.tile([P, P], f32, tag="mm" if pool is ps_mm else "sc",
                             name="v_ps")
            for dc in range(DCH):
                nc.tensor.matmul(
                    v_ps[:], hT_sb[:, dc, tsl], wv_sb[:, dc, :],
                    start=(dc == 0), stop=(dc == DCH - 1))
            nc.vector.tensor_add(
                v_sb[:, tt, 0:HPC, 0:DH],
                v_ps[:].rearrange("p (h j) -> p h j", j=DH),
                bvb_sb[:].rearrange("p (h j) -> p h j", j=DH))

        def proj_thunks(bi, pools=(ps_mm,)):
            th = []
            k = [0]

            def nxt():
                p = pools[k[0] % len(pools)]
                k[0] += 1
                return p
            for w_sb, b_sb, dst in ((wq_sb, bq_sb, qT_sb), (wk_sb, bk_sb, kT_sb)):
                for ci in range(2 * bi, 2 * bi + 2):
                    th.append(lambda w=w_sb, b=b_sb, d=dst, c=ci:
                              emit_qk_chunk(w, b, d, c, nxt()))
            for tt in range(8 * bi, 8 * bi + 8):
                th.append(lambda t=tt: emit_v_tile(t, nxt()))
            return th

        def emit_av_mm(bi, hl, pps, ctx_tiles, uc):
            for c2 in range(2):
                nc.tensor.matmul(
                    ctx_tiles[c2][:],
                    v_sb[:, bi * 8 + uc, hl, :],
                    pps[uc][:, c2 * CH:(c2 + 1) * CH],
                    start=(uc == 0), stop=(uc == DCH - 1))

        def emit_norm(bi, hl, ctx_tiles):
            for c2 in range(2):
                ctx_ps = ctx_tiles[c2]
                csl = slice(c2 * CH, (c2 + 1) * CH)
                den_sb = small.tile([DH + 1, CH], f32, name="den_sb")
                nc.vector.tensor_copy(den_sb[DH:DH + 1, :], ctx_ps[DH:DH + 1, :])
                den0_sb = small.tile([1, CH], f32, name="den0_sb")
                nc.sync.dma_start(den0_sb[:], den_sb[DH:DH + 1, :])
                rcp_sb = small.tile([1, CH], f32, name="rcp_sb")
                nc.vector.reciprocal_approx_fast(rcp_sb[:], den0_sb[:])
                rcpb_sb = small.tile([DH, CH], f32, name="rcpb_sb")
                nc.gpsimd.partition_broadcast(rcpb_sb[:], rcp_sb[:])
                o_sb = out_pool.tile([DH, CH], f32, name="o_sb")
                nc.vector.tensor_mul(o_sb[:], ctx_ps[0:DH, :], rcpb_sb[:])
                nc.sync.dma_start(out_d.ap()[bi, hl, :, csl], o_sb[:])

        def new_ctx_tiles():
            return [ps_ctx.tile([DH + 1, CH], f32, tag="ctx", name="ctx_ps")
                    for _ in range(2)]

        # Software pipeline over batches. Per batch: both heads' score
        # matmuls are emitted adjacently (tile_position row packing runs the
        # two K=64 matmuls concurrently); AV of the previous batch's head 0
        # is interleaved per-uc, head 1 runs dense after the loop;
        # projections of batch bi+1 fill the remaining PE slack.
        for th in proj_thunks(0, pools=(ps_mm, ps_sc)):
            th()

        prev = None          # (bi, [pps_h0, pps_h1])
        queue = []
        for bi in range(B):
            if bi + 1 < B:
                while queue:
                    queue.pop(0)()
                queue = proj_thunks(bi + 1)
            pps = [[], []]
            ctxA = new_ctx_tiles() if prev is not None else None
            for uc in range(DCH):
                usl = slice(bi * S + uc * P, bi * S + (uc + 1) * P)
                sc_tiles = []
                for hl in range(HPC):
                    sc_tiles.append(
                        ps_sc.tile([P, S], f32, tag="sc", name="sc_ps"))
                for c2 in range(2):
                    qsl = slice(bi * S + c2 * CH, bi * S + (c2 + 1) * CH)
                    for hl in range(HPC):
                        hsl = slice(hl * DH, (hl + 1) * DH)
                        nc.tensor.matmul(
                            sc_tiles[hl][:, c2 * CH:(c2 + 1) * CH],
                            kT_sb[hsl, usl], qT_sb[hsl, qsl],
                            start=True, stop=True)
                for hl in range(HPC):
                    pp = probs_pool.tile([P, S], bf16, name="pp")
                    nc.scalar.activation(
                        pp[:], sc_tiles[hl][:], AF.Exp,
                        bias=mask_bias(uc, bi), scale=scale)
                    pps[hl].append(pp)
                if prev is not None:
                    emit_av_mm(prev[0], 0, prev[1][0], ctxA, uc)
                if queue and uc < 6:
                    queue.pop(0)()
            if prev is not None:
                pbi, ppps = prev
                emit_norm(pbi, 0, ctxA)
                ctxB = new_ctx_tiles()
                for uc in range(DCH):
                    emit_av_mm(pbi, 1, ppps[1], ctxB, uc)
                emit_norm(pbi, 1, ctxB)
            prev = (bi, pps)
        # epilogue: last batch's attention output
        while queue:
            queue.pop(0)()
        pbi, ppps = prev
        for hl in range(HPC):
            ctxE = new_ctx_tiles()
            for uc in range(DCH):
                emit_av_mm(pbi, hl, ppps[hl], ctxE, uc)
            emit_norm(pbi, hl, ctxE)

    nc.compile()
    return nc


def _get_nc():
    global _compiled_nc
    if _compiled_nc is None:
        _compiled_nc = _build()
    return _compiled_nc


def prepare_in_maps(hidden_states, attention_mask, Wq, bq, Wk, bk, Wv, bv):
    bf16 = ml_dtypes.bfloat16

    hs = np.asarray(hidden_states, dtype=np.float32)            # [S, B, D]
    hT = np.ascontiguousarray(hs.transpose(2, 1, 0).reshape(D, BS)).astype(bf16)
    maskT = np.ascontiguousarray(
        np.asarray(attention_mask, dtype=np.float32).reshape(B, S).T)
    Wq = np.asarray(Wq, dtype=np.float32)
    Wk = np.asarray(Wk, dtype=np.float32)
    Wv = np.asarray(Wv, dtype=np.float32)
    bq = np.asarray(bq, dtype=np.float32)
    bk = np.asarray(bk, dtype=np.float32)
    bv = np.asarray(bv, dtype=np.float32)

    # maskT packed as [p, uc, b] -> [128, 32]
    mask_pk = maskT.reshape(DCH, P, B).transpose(1, 0, 2).reshape(P, DCH * B)
    in_maps = []
    for c in range(NCORES):
        sl = slice(P * c, P * (c + 1))
        misc = np.empty((P, 2 + P + DCH * B), dtype=np.float32)
        misc[:, 0] = bq[sl]
        misc[:, 1] = bk[sl]
        misc[:, 2:2 + P] = np.broadcast_to(bv[sl][None, :], (P, P))
        misc[:, 2 + P:] = mask_pk
        in_maps.append({
            "hT": hT,
            "wqT": np.ascontiguousarray(Wq[sl, :].T).astype(bf16),
            "wkT": np.ascontiguousarray(Wk[sl, :].T).astype(bf16),
            "wvT": np.ascontiguousarray(Wv[sl, :].T).astype(bf16),
            "misc": misc,
        })
    return in_maps


def kernel(hidden_states, attention_mask, Wq, bq, Wk, bk, Wv, bv):
    global last_exec_time_ns, last_results
    from concourse.bass_utils import run_bass_kernel_spmd

    nc = _get_nc()
    in_maps = prepare_in_maps(hidden_states, attention_mask,
                              Wq, bq, Wk, bk, Wv, bv)

    trace = bool(int(os.environ.get("KERNEL_TRACE", "0")))
    tmpdir = os.environ.get("KERNEL_TRACE_DIR") or None
    res = run_bass_kernel_spmd(nc, in_maps, core_ids=list(range(NCORES)),
                               trace=trace, tmpdir=tmpdir)
    last_exec_time_ns = res.exec_time_ns
    last_results = res

    # gather: per-core out [B, HPC, DH, S] -> full [S, B, D]
    outs = np.stack([np.asarray(res.results[c]["out"]) for c in range(NCORES)],
                    axis=0)                                     # [C, B, HPC, DH, S]
    full = outs.transpose(4, 1, 0, 2, 3).reshape(S, B, D)       # s, b, (c, hl, j)
    return np.ascontiguousarray(full.astype(np.float32))
